# revision 3
# baseline (speedup 1.0000x reference)
"""Trainium2 Bass kernel for an 8-head MultiHeadAttention (b=8, s=1024, d=512).

Sharding: pure data-parallel over batch -- each of the 8 NeuronCores runs the
full attention for one batch element. No collectives.

v2 design (matmul operands bf16, accumulate fp32):
  x and the four weights reach SBUF pre-transposed and pre-cast with zero
  PE/DVE/ACT work: gpsimd SWDGE cast-DMAs (f32->bf16) land them in DRAM
  scratch, then HWDGE xbar DMA-transposes deliver x^T / w^T straight into
  SBUF (verified semantics: out[p,c,f] = in[f, c*128+p]).  The mask keeps
  the v1 path (f32 column strips on sync/scalar + PE transposes) so the
  gpsimd cast queue stays short.

  Q^T[hd,s] = wq^T.T @ x^T   (scale 1/8 + bias folded into the PSUM drain)
  K^T[hd,s] = wk^T.T @ x^T
  V[s,hd]   = x^T.T @ wv^T   (+ bv via rank-1 ones matmul; ones col per head)
  S^T[k,q]  = K_h^T.T @ Q_h^T  -- head-pair concurrent via PE 64-row tiling.
  P^T       = exp(S^T) * (1-mask)^T  (exp on ACT; om from PSUM via DVE)
  O^T_h[65,q] = V_aug.T @ P^T  (row 64 = softmax denominator via ones col)
  normalize: recip via ACT ln->exp(-x) (activation tables patched so exp/ln
             share one set => zero table switches), indicator-matmul
             broadcast, in-place DVE mul.
  out[q,d]  = O^T.T @ wo^T + bo (bo via rank-1 ones matmul)

Schedule: software-pipelined at kc granularity like v1, but the ramp is
DMA-only: a gated dense warm-up matmul burst holds the PE HAM clock-gate
open until the first projection; chunk-0 q/k projections interleave at rr
granularity so the PE never idles >3.4us while x-half/wk transposes land.
PSUM banks: psc [128,1024]x3 + ppv [128,512]x2 = 8.
"""

import numpy as np

P = 128
S = 1024  # sequence length
D = 512  # d_model
H = 8  # heads
DK = 64  # head dim
CH = D // P  # 4 hd/dmodel chunks
ST = S // P  # 8 seq tiles
NCORES = 8

# mask-mul strips handled by gpsimd for pairs 1-3 (by kc index)
GP_MUL_KC = (6, 7)

_CACHE = {}


def _patch_act_tables():
    """Force every activation function to resolve to the combined
    natural_log_exp_and_others set so exp and ln share one table load."""
    import concourse.hw_specs as hw_specs
    import concourse.bacc as bacc_mod

    if getattr(hw_specs, "_mha_patched", False):
        return
    _orig = hw_specs.get_activation_tables

    def _patched(arch):
        t = _orig(arch)
        return {
            name: (fns if name == "natural_log_exp_and_others" else set())
            for name, fns in t.items()
        }

    hw_specs.get_activation_tables = _patched
    hw_specs._mha_patched = True
    if hasattr(bacc_mod, "get_activation_tables"):
        bacc_mod.get_activation_tables = _patched


def _build():
    _patch_act_tables()
    import concourse.bacc as bacc
    import concourse.mybir as mybir
    import concourse.tile as tile
    from concourse.masks import make_identity

    f32 = mybir.dt.float32
    mmdt = mybir.dt.bfloat16
    AF = mybir.ActivationFunctionType
    OP = mybir.AluOpType

    nc = bacc.Bacc(None, target_bir_lowering=False, debug=False)

    x_t = nc.dram_tensor("x", [S, D], f32, kind="ExternalInput")
    mask_t = nc.dram_tensor("mask", [S, S], f32, kind="ExternalInput")
    wq_t = nc.dram_tensor("wq", [D, D], f32, kind="ExternalInput")
    wk_t = nc.dram_tensor("wk", [D, D], f32, kind="ExternalInput")
    wv_t = nc.dram_tensor("wv", [D, D], f32, kind="ExternalInput")
    wo_t = nc.dram_tensor("wo", [D, D], f32, kind="ExternalInput")
    bq_t = nc.dram_tensor("bq", [D], f32, kind="ExternalInput")
    bk_t = nc.dram_tensor("bk", [D], f32, kind="ExternalInput")
    bv_t = nc.dram_tensor("bv", [D], f32, kind="ExternalInput")
    bo_t = nc.dram_tensor("bo", [D], f32, kind="ExternalInput")
    out_t = nc.dram_tensor("out", [S, D], f32, kind="ExternalOutput")

    with tile.TileContext(nc) as tc:
        with (
            tc.tile_pool(name="dram", space="DRAM", bufs=1) as dp,
            tc.tile_pool(name="persist", bufs=1) as pp,
            tc.tile_pool(name="mstage", bufs=4) as mstage,
            tc.tile_pool(name="ptp", bufs=4) as ptp,
            tc.tile_pool(name="nrm", bufs=2) as nrm,
            tc.tile_pool(name="fin", bufs=3) as fpool,
            tc.tile_pool(name="psc", bufs=3, space="PSUM") as psc,
            tc.tile_pool(name="ppv", bufs=2, space="PSUM") as ppv,
        ):
            # ---- constants ----
            ident = pp.tile([P, P], f32, name="id", tag="id")
            make_identity(nc, ident[:])
            ones_f32 = pp.tile([P, P], f32, name="ones_f32", tag="ones_f32")
            nc.vector.memset(ones_f32[:], 1.0)
            ones_sb = pp.tile([1, P], mmdt, name="ones", tag="ones")
            nc.vector.tensor_copy(ones_sb[:], ones_f32[0:1, :])
            ones512 = pp.tile([1, 512], mmdt, name="ones512", tag="ones512")
            nc.vector.memset(ones512[:], 1.0)
            # hoist the (single) activation table load to t=0: first ACT
            # instruction in program order is this dummy
            warmact = pp.tile([1, 2], f32, name="warmact", tag="warmact")
            nc.scalar.activation(warmact[:], ones_f32[0:1, 0:2], AF.Identity)
            # indicator for the recip broadcast: denominator slot i lives on
            # partition 32*i; for j-slice, out rows 0:64 take slot 2j and
            # rows 64:128 take slot 2j+1
            e4 = pp.tile([P, 2 * P], mmdt, name="e4", tag="e4")
            nc.vector.memset(e4[:], 0.0)
            for j in range(2):
                nc.vector.memset(
                    e4[32 * 2 * j : 32 * 2 * j + 1, j * P : j * P + 64], 1.0
                )
                nc.vector.memset(
                    e4[32 * (2 * j + 1) : 32 * (2 * j + 1) + 1,
                       j * P + 64 : (j + 1) * P], 1.0
                )

            bq_sb = pp.tile([P, CH], f32, name="bq", tag="bq")
            bk_sb = pp.tile([P, CH], f32, name="bk", tag="bk")
            qbias_sb = pp.tile([P, CH], f32, name="qbias", tag="qbias")
            bv_row = pp.tile([1, D], f32, name="bvr", tag="bvr")
            bo_row = pp.tile([1, D], f32, name="bor", tag="bor")
            bv_bf = pp.tile([1, D], mmdt, name="bvb", tag="bvb")
            bo_bf = pp.tile([1, D], mmdt, name="bob", tag="bob")

            # ---- DRAM scratch (bf16 casts) ----
            x_bf = dp.tile([S, D], mmdt, name="xbf", tag="xbf")
            w_bf = {
                n: dp.tile([D, D], mmdt, name="wbf", tag=f"wbf{n}")
                for n in ("wq", "wk", "wv", "wo")
            }

            # ---- gpsimd cast-DMAs (f32 -> bf16); transfer order = issue
            # order: wq first (gates the warm-up + first projection), x in
            # two column halves so chunk 0/1 transposes start early ----
            nc.gpsimd.dma_start(out=w_bf["wq"][:], in_=wq_t[:])
            nc.gpsimd.dma_start(out=x_bf[:, 0:256], in_=x_t[:, 0:256])
            nc.gpsimd.dma_start(out=w_bf["wk"][:], in_=wk_t[:])
            nc.gpsimd.dma_start(out=x_bf[:, 256:512], in_=x_t[:, 256:512])
            nc.gpsimd.dma_start(out=w_bf["wv"][:], in_=wv_t[:])
            nc.gpsimd.dma_start(out=w_bf["wo"][:], in_=wo_t[:])

            # ---- sync (SP) HWDGE queue ----
            nc.sync.dma_start(out=bq_sb[:], in_=bq_t[:].rearrange("(c p) -> p c", p=P))
            nc.sync.dma_start(out=bk_sb[:], in_=bk_t[:].rearrange("(c p) -> p c", p=P))
            nc.sync.dma_start(out=bv_row[:], in_=bv_t[None, :])
            nc.sync.dma_start(out=bo_row[:], in_=bo_t[None, :])
            nc.vector.tensor_scalar_mul(qbias_sb[:], bq_sb[:], 0.125)
            nc.vector.tensor_copy(bv_bf[:], bv_row[:])
            nc.vector.tensor_copy(bo_bf[:], bo_row[:])

            # mask column strips (f32, classic path): even strips on sync,
            # odd strips on scalar, interleaved with the xbar transposes so
            # no queue blocks a ramp-critical transfer
            msk = {}

            def msk_dma(kc, eng):
                m = mstage.tile([P, ST, P], f32, name="msk", tag="msk")
                eng.dma_start(
                    out=m[:],
                    in_=mask_t[:, kc * P : (kc + 1) * P].rearrange(
                        "(i p) k -> p i k", p=P
                    ),
                )
                msk[kc] = m

            msk_dma(0, nc.sync)
            msk_dma(2, nc.sync)
            # warm-up gate: cannot start before the wq cast has landed
            gate = pp.tile([1, P], mmdt, name="gate", tag="gate")
            nc.sync.dma_start(out=gate[:], in_=w_bf["wq"][0:1, 0:P])

            xT = pp.tile([P, CH, S], mmdt, name="xT", tag="xT")
            nc.sync.dma_start(out=xT[:, 0:2, :], in_=x_bf[:, 0:256], transpose=True)
            nc.sync.dma_start(out=xT[:, 2:4, :], in_=x_bf[:, 256:512], transpose=True)
            wT = {}
            for n in ("wq", "wk", "wv", "wo"):
                wT[n] = pp.tile([P, CH, D], mmdt, name="T", tag=f"T{n}")
            nc.sync.dma_start(out=wT["wv"][:], in_=w_bf["wv"][:], transpose=True)
            msk_dma(4, nc.sync)
            msk_dma(6, nc.sync)
            nc.sync.dma_start(out=wT["wo"][:], in_=w_bf["wo"][:], transpose=True)

            # ---- scalar (ACT) HWDGE queue ----
            msk_dma(1, nc.scalar)
            nc.scalar.dma_start(out=wT["wq"][:], in_=w_bf["wq"][:], transpose=True)
            nc.scalar.dma_start(out=wT["wk"][:], in_=w_bf["wk"][:], transpose=True)
            msk_dma(3, nc.scalar)
            msk_dma(5, nc.scalar)
            msk_dma(7, nc.scalar)

            # ---- gated PE warm-up: dense MM burst so the HAM clock is warm
            # when the first projection becomes ready ----
            def warm(n=2, gated=True):
                jp = ppv.tile([P, 512], f32, name="pv", tag="pv")
                lhs = gate if gated else ones_sb
                for _ in range(n):
                    nc.tensor.matmul(
                        jp[0:64, 0:512], lhs[:, 0:64], ones512[:],
                        start=True, stop=True,
                    )

            warm(12)

            # ---- projections Q^T, K^T ----
            qT = pp.tile([P, CH, S], mmdt, name="qT", tag="qT")
            kT = pp.tile([P, CH, S], mmdt, name="kT", tag="kT")

            omT = pp.tile([P, ST, S], mmdt, name="omT", tag="omT")

            def build_om(kc):
                ps = psc.tile([P, S], f32, name="ps", tag="ps")
                for qi in range(ST):
                    nc.tensor.transpose(
                        ps[:, qi * P : (qi + 1) * P], msk[kc][:, qi, :], ident[:]
                    )
                nc.vector.tensor_scalar(
                    omT[:, kc, :], ps[:], -1.0, 1.0, op0=OP.mult, op1=OP.add
                )

            def proj_drain(c, dst, bias, scale, on_act, ps):
                if on_act:
                    nc.scalar.activation(
                        dst[:, c, :], ps[:], AF.Identity,
                        bias=bias[:, c : c + 1], scale=scale,
                    )
                else:
                    nc.vector.tensor_scalar(
                        dst[:, c, :], ps[:], scale, bias[:, c : c + 1],
                        op0=OP.mult, op1=OP.add,
                    )

            def proj_qk_dst(c, dst, wname, bias, scale, on_act):
                ps = psc.tile([P, S], f32, name="ps", tag="ps")
                for j in range(2):
                    for rr in range(CH):
                        nc.tensor.matmul(
                            ps[:, j * 512 : (j + 1) * 512],
                            wT[wname][:, rr, c * P : (c + 1) * P],
                            xT[:, rr, j * 512 : (j + 1) * 512],
                            start=(rr == 0),
                            stop=(rr == CH - 1),
                        )
                proj_drain(c, dst, bias, scale, on_act, ps)

            # chunk-0 q/k interleaved at rr granularity: rr 0/1 matmuls run
            # as soon as the first x half + each w transpose lands, keeping
            # the PE gap-free through the ramp (om 0/1 transposes fill the
            # DMA waits)
            psq = psc.tile([P, S], f32, name="ps", tag="ps")
            psk = psc.tile([P, S], f32, name="ps", tag="ps")

            def proj0_part(ps, wname, rrs, start, stop):
                for j in range(2):
                    for rr in rrs:
                        nc.tensor.matmul(
                            ps[:, j * 512 : (j + 1) * 512],
                            wT[wname][:, rr, 0:P],
                            xT[:, rr, j * 512 : (j + 1) * 512],
                            start=(rr == rrs[0] and start),
                            stop=(rr == rrs[-1] and stop),
                        )

            proj0_part(psq, "wq", [0, 1], True, False)
            build_om(0)
            proj0_part(psk, "wk", [0, 1], True, False)
            build_om(1)
            proj0_part(psq, "wq", [2, 3], False, True)
            proj0_part(psk, "wk", [2, 3], False, True)
            proj_drain(0, qT, qbias_sb, 0.125, True, psq)
            proj_drain(0, kT, bk_sb, 1.0, True, psk)

            # ---- persistent attention state ----
            v_sb = pp.tile([P, ST, H * 65], mmdt, name="v", tag="v")
            oT = pp.tile([P, CH, S], mmdt, name="oT", tag="oT")

            def proj_v_unit(i):
                ps = ppv.tile([P, 512], f32, name="pv", tag="pv")
                for rr in range(CH):
                    nc.tensor.matmul(
                        ps[:],
                        xT[:, rr, i * P : (i + 1) * P],
                        wT["wv"][:, rr, :],
                        start=(rr == 0),
                        stop=False,
                    )
                # rank-1 bias: out[s, hd] += 1 * bv[hd]
                nc.tensor.matmul(
                    ps[:], ones_sb[:, 0:P], bv_bf[:], start=False, stop=True
                )
                nc.vector.tensor_copy(
                    v_sb[:, i, :].rearrange("p (h e) -> p h e", e=65)[:, :, 0:64],
                    ps[:].rearrange("p (h e) -> p h e", e=64),
                )

            # partial output projection: chunks 0-2 plus bias (rank-1),
            # accumulated into bf16 SBUF during S(3) so the tail needs one
            # matmul per q-tile
            f_acc = pp.tile([P, ST, 512], mmdt, name="facc", tag="facc")
            finals = []

            def partial_qt(qt):
                if qt % 2 == 0:
                    finals.append(psc.tile([P, S], f32, name="ps", tag="ps"))
                half = finals[-1][:, (qt % 2) * 512 : (qt % 2) * 512 + 512]
                for cc in range(CH - 1):
                    nc.tensor.matmul(
                        half,
                        oT[:, cc, qt * P : (qt + 1) * P],
                        wT["wo"][:, cc, :],
                        start=(cc == 0),
                        stop=False,
                    )
                nc.tensor.matmul(
                    half, ones_sb[:, 0:P], bo_bf[:], start=False, stop=True
                )
                nc.vector.tensor_copy(f_acc[:, qt, :], half)

            # ---- pipelined attention ----
            from collections import deque

            pts = {}
            pvs = {}
            dns = {}
            pvq = {}

            def scores_unit(p, kc):
                c = p
                ptA, ptB = pts[2 * p], pts[2 * p + 1]
                kA = kT[0:64, c, kc * P : (kc + 1) * P]
                kB = kT[64:128, c, kc * P : (kc + 1) * P]
                psA = psc.tile([P, S], f32, name="ps", tag="ps")
                psB = psc.tile([P, S], f32, name="ps", tag="ps")
                for j in range(2):
                    nc.tensor.matmul(
                        psA[:, j * 512 : (j + 1) * 512],
                        kA, qT[0:64, c, j * 512 : (j + 1) * 512],
                        start=True, stop=True,
                    )
                    nc.tensor.matmul(
                        psB[:, j * 512 : (j + 1) * 512],
                        kB, qT[64:128, c, j * 512 : (j + 1) * 512],
                        start=True, stop=True,
                    )
                nc.scalar.activation(ptA[:, kc, :], psA[:], AF.Exp)
                nc.scalar.activation(ptB[:, kc, :], psB[:], AF.Exp)
                if p == 0:
                    # S(0): DVE also builds om strips; split the pair's muls
                    nc.gpsimd.tensor_mul(ptA[:, kc, :], ptA[:, kc, :], omT[:, kc, :])
                    nc.vector.tensor_mul(ptB[:, kc, :], ptB[:, kc, :], omT[:, kc, :])
                else:
                    eng = nc.gpsimd if kc in GP_MUL_KC else nc.vector
                    eng.tensor_mul(ptA[:, kc, :], ptA[:, kc, :], omT[:, kc, :])
                    eng.tensor_mul(ptB[:, kc, :], ptB[:, kc, :], omT[:, kc, :])

            def pv_start(p, j):
                pvA = ppv.tile([P, 512], f32, name="pv", tag="pv")
                pvB = ppv.tile([P, 512], f32, name="pv", tag="pv")
                pvs[(p, j)] = (pvA, pvB)

            def pv_steps(p, j, kcs):
                hA, hB = 2 * p, 2 * p + 1
                ptA, ptB = pts[hA], pts[hB]
                vA = v_sb[:].rearrange("p i (h e) -> p i h e", e=65)[:, :, hA, :]
                vB = v_sb[:].rearrange("p i (h e) -> p i h e", e=65)[:, :, hB, :]
                jsl = slice(j * 512, (j + 1) * 512)
                pvA, pvB = pvs[(p, j)]
                for kc in kcs:
                    st = kc == 0
                    sp = kc == ST - 1
                    nc.tensor.matmul(
                        pvA[0:65, :], vA[:, kc, :], ptA[:, kc, jsl],
                        start=st, stop=sp,
                    )
                    nc.tensor.matmul(
                        pvB[0:65, :], vB[:, kc, :], ptB[:, kc, jsl],
                        start=st, stop=sp,
                    )

            def pv_drain(p, j):
                c = p
                hA, hB = 2 * p, 2 * p + 1
                jsl = slice(j * 512, (j + 1) * 512)
                pvA, pvB = pvs.pop((p, j))
                dn = dns[p]
                for idx, (h, pv) in enumerate(((hA, pvA), (hB, pvB))):
                    off = 64 * (h % 2)
                    slot = 32 * (2 * j + idx)
                    nc.vector.tensor_copy(oT[off : off + 64, c, jsl], pv[0:64, :])
                    nc.vector.tensor_copy(dn[slot : slot + 1, :], pv[64:65, :])

            def pv_norm(p, then=None):
                # reciprocal of the pair's 4 denominator rows via ACT
                # ln -> exp(-x); non-slot lanes hold 1.0 -> stay 1.0
                c = p
                hA, hB = 2 * p, 2 * p + 1
                dn = dns.pop(p)
                lnd = nrm.tile([P, 512], f32, name="lnd", tag="lnd")
                nc.scalar.activation(lnd[:], dn[:], AF.Ln)
                rb4 = nrm.tile([P, 512], mmdt, name="rb4", tag="rb4")
                nc.scalar.activation(rb4[:], lnd[:], AF.Exp, scale=-1.0)
                for j in range(2):
                    jsl = slice(j * 512, (j + 1) * 512)
                    bp = psc.tile([P, S], f32, name="ps", tag="ps")
                    nc.tensor.matmul(
                        bp[:, 0:512], e4[:, j * P : (j + 1) * P], rb4[:],
                        start=True, stop=True,
                    )
                    for idx, h in enumerate((hA, hB)):
                        off = 64 * (h % 2)
                        osl = oT[off : off + 64, c, jsl]
                        nc.vector.tensor_mul(
                            osl, osl, bp[64 * idx : 64 * idx + 64, 0:512]
                        )
                    if then is not None:
                        then(j)

            def new_pair(p):
                pts[2 * p] = ptp.tile([P, ST, S], mmdt, name="pt", tag="pt")
                pts[2 * p + 1] = ptp.tile([P, ST, S], mmdt, name="pt", tag="pt")
                dns[p] = nrm.tile([P, 512], f32, name="dn", tag="dn")
                nc.vector.memset(dns[p][:], 1.0)
                # j-sequential: one (pair, j) accumulator group at a time
                # (the 2-deep ppv ring holds exactly one A/B group)
                pvq[p] = deque((j, kc) for j in (0, 1) for kc in range(ST))

            # ---- S(0): om + scores(0) + V projection ----
            new_pair(0)
            nc.vector.tensor_copy(
                v_sb[:].rearrange("p i (h e) -> p i h e", e=65)[:, :, :, 64],
                ones_f32[:, 0 : ST * H].rearrange("p (i h) -> p i h", h=H),
            )
            VPROJ = {4: (0, 1), 5: (2, 3), 6: (4, 5), 7: (6, 7)}
            for kc in range(ST):
                if kc >= 2:
                    build_om(kc)
                scores_unit(0, kc)
                # chunk-1 QK projections ride the first two slots (S(1)
                # needs them; S(0) only needed chunk 0)
                if kc == 0:
                    proj_qk_dst(1, qT, "wq", qbias_sb, 0.125, True)
                if kc == 1:
                    proj_qk_dst(1, kT, "wk", bk_sb, 1.0, True)
                for i in VPROJ.get(kc, ()):
                    proj_v_unit(i)

            # ---- S(1..3) + tail: PV-step scheduler ----
            started = set()
            jdone = {}

            def pump(order, avail, budget):
                for p_, limit in order:
                    q = pvq.get(p_)
                    if p_ > 0 and (p_ - 1) in pvq:
                        continue  # previous pair still owns the ppv ring
                    while q and budget > 0 and limit > 0:
                        j, kc = q[0]
                        if kc > avail.get(p_, ST):
                            break
                        q.popleft()
                        if (p_, j) not in started:
                            started.add((p_, j))
                            pv_start(p_, j)
                        pv_steps(p_, j, [kc])
                        jdone[(p_, j)] = jdone.get((p_, j), 0) + 1
                        if jdone[(p_, j)] == ST:
                            pv_drain(p_, j)
                            if jdone.get((p_, 1)) == ST:
                                del pvq[p_]
                                # pair 3's norm is interleaved with the
                                # output projection in the tail
                                if p_ != H // 2 - 1:
                                    pv_norm(p_)
                                break
                        budget -= 1
                        limit -= 1

            bg = {
                (1, 3): lambda: proj_qk_dst(2, qT, "wq", qbias_sb, 0.125, False),
                (1, 6): lambda: proj_qk_dst(2, kT, "wk", bk_sb, 1.0, False),
                (2, 2): lambda: proj_qk_dst(3, qT, "wq", qbias_sb, 0.125, False),
                (2, 5): lambda: proj_qk_dst(3, kT, "wk", bk_sb, 1.0, False),
            }

            for p in (1, 2, 3):
                new_pair(p)
                for kc in range(ST):
                    scores_unit(p, kc)
                    if (p, kc) in bg:
                        bg[(p, kc)]()
                    pump([(p - 1, 2)], {p - 1: ST}, 2)

            # tail: pair-3 PV pumped in chunks, interleaved with the
            # partial output projections (chunks 0-2 are normalized by now)
            for qtp in range(4):
                pump([(3, 4)], {3: ST}, 4)
                partial_qt(2 * qtp)
                partial_qt(2 * qtp + 1)
            while pvq:
                pump([(0, 8), (1, 8), (2, 8), (3, 8)], {0: ST, 1: ST, 2: ST, 3: ST}, 16)

            warm(6, gated=False)

            def tail_finals(j):
                # oT chunk 3 cols j*512.. cover q-tiles 4j..4j+3: one matmul
                # each on top of the precomputed partial
                for qt in range(4 * j, 4 * j + 4):
                    if qt % 2 == 0:
                        finals.append(psc.tile([P, S], f32, name="ps", tag="ps"))
                    half = finals[-1][:, (qt % 2) * 512 : (qt % 2) * 512 + 512]
                    nc.tensor.matmul(
                        half,
                        oT[:, 3, qt * P : (qt + 1) * P],
                        wT["wo"][:, 3, :],
                        start=True, stop=True,
                    )
                    ft = fpool.tile([P, 512], f32, name="fin", tag="fin")
                    nc.vector.tensor_add(ft[:], half, f_acc[:, qt, :])
                    # alternate output DMAs across the SP and ACT queues to
                    # halve tail issue serialization
                    eng = nc.sync if qt % 2 == 0 else nc.scalar
                    eng.dma_start(out=out_t[qt * P : (qt + 1) * P, :], in_=ft[:])
                warm(2, gated=False)

            pv_norm(3, then=tail_finals)

    nc.compile()
    return nc


def _get_nc():
    if "nc" not in _CACHE:
        _CACHE["nc"] = _build()
    return _CACHE["nc"]


def run(inputs, trace=False, **kw):
    from concourse.bass_utils import run_bass_kernel_spmd

    nc = _get_nc()
    f = np.float32
    in_maps = [
        {
            "x": np.ascontiguousarray(inputs["inputs"][i], dtype=f),
            "mask": np.ascontiguousarray(inputs["mask"][i], dtype=f),
            "wq": np.ascontiguousarray(inputs["wq"], dtype=f),
            "wk": np.ascontiguousarray(inputs["wk"], dtype=f),
            "wv": np.ascontiguousarray(inputs["wv"], dtype=f),
            "wo": np.ascontiguousarray(inputs["wo"], dtype=f),
            "bq": np.ascontiguousarray(inputs["bq"], dtype=f),
            "bk": np.ascontiguousarray(inputs["bk"], dtype=f),
            "bv": np.ascontiguousarray(inputs["bv"], dtype=f),
            "bo": np.ascontiguousarray(inputs["bo"], dtype=f),
        }
        for i in range(NCORES)
    ]
    res = run_bass_kernel_spmd(nc, in_maps, list(range(NCORES)), trace=trace, **kw)
    out = np.stack(
        [np.asarray(res.results[i]["out"], dtype=np.float32) for i in range(NCORES)],
        axis=0,
    )
    return out, res


def kernel(**inputs):
    out, _ = run(inputs)
    return out
